# revision 23
# baseline (speedup 1.0000x reference)
"""Equivariant LayerNorm (128x0e + 64x1o + 32x2e) Trainium2 Bass kernel, v4.

Sharding: pure data parallel over 8 NeuronCores, 32768 rows each.

Layout per core: tiles of 128*B rows; SBUF tile [128 partitions, B*480]
(row-block b at free offset b*480; p-major DRAM blocking gives contiguous
15KB per-partition DMA runs).

One-pass stats with negated means:
  nm = -S1/d,  var = S2/d - nm^2 (fused STT),  inv = rsqrt(var + eps)
  out = (x + nm_b) * inv_b          (v1/v2, broadcast per segment)
  out = (x*inv + nm*inv)*w + bias   (scal block, per-b ScalarE activation)

The emission order software-pipelines three tiles: loads prefetch two
tiles ahead; tile i-1's stats head (nm -> msq -> var -> rsqrt) is emitted
before tile i's sums so ScalarE/GPSIMD fill while VectorE runs the sums,
and tile i-1's scal tail (wv/ba) lands after the sums when ScalarE's
per-b activations have finished.

Engine split (measured: V 1.08 ns/el TT/reduce + ~140 fix, 0.56 ts f32;
S 0.91 ns/el + ~300 fix; G ~1.9-2.6 ns/el):
  SP/HWDGE : load x, store out
  VectorE  : v1 sums via TT-add trees (2 in-el/cyc), v2+scal sums via
             reduce, nm ts ops, fused var STT, v2 centering, scal wv/ba
  ScalarE  : full-row Square, msq, single fused Rsqrt, per-b scal
             normalize activations
  GPSIMD   : v1 centering+scale, v2 scale
"""

import sys

import numpy as np

try:
    import concourse  # noqa: F401
except ImportError:  # pragma: no cover
    sys.path.insert(0, "/opt/trn_rl_repo")

from contextlib import ExitStack

import concourse.bacc as bacc
import concourse.bass as bass
import concourse.mybir as mybir
import concourse.tile as tile
from concourse.bass_utils import run_bass_kernel_spmd

F32 = mybir.dt.float32
AF = mybir.ActivationFunctionType
AXX = mybir.AxisListType.X
ALU = mybir.AluOpType

N = 262144
DIM = 480
S = 128
G1, D1 = 64, 3
G2, D2 = 32, 5
V12 = G1 * D1 + G2 * D2  # 352
NSEG = G1 + G2 + 1  # 97
EPS = 1e-5

N_CORES = 8
ROWS = N // N_CORES  # 32768
B = 8
TILE_ROWS = 128 * B

# engine knobs
ENG_C1 = "gpsimd"  # c1 = x + nm1_b
ENG_O1 = "gpsimd"  # o1 = c1 * inv1_b
ENG_C2 = "vector"  # c2 = x + nm2_b
ENG_O2 = "gpsimd"  # o2 = c2 * inv2_b
ENG_WM = "vector"  # scal wv = t * w_b
ENG_BA = "vector"  # scal out = wv + bias_b


def _rsqrt(nc, out_ap, in_ap, scale, bias):
    """out = Rsqrt(in*scale + bias) on ScalarE, immediate scale/bias.
    bass rejects Rsqrt for accuracy; measured ~4e-5 max rel err here,
    far below the 2e-2 tolerance."""
    eng = nc.scalar
    return eng.add_instruction(
        mybir.InstActivation(
            name=nc.get_next_instruction_name(),
            func=AF.Rsqrt,
            ins=[
                eng.lower_ap(in_ap),
                mybir.ImmediateValue(dtype=F32, value=bias),
                mybir.ImmediateValue(dtype=F32, value=scale),
                mybir.ImmediateValue(dtype=F32, value=0.0),
            ],
            outs=[eng.lower_ap(out_ap)],
        )
    )


def build_nc(rows=ROWS, b_blocks=B):
    nc = bacc.Bacc("TRN2", target_bir_lowering=False, debug=False)
    Bb = b_blocks
    trows = 128 * Bb
    assert rows % trows == 0
    ntiles = rows // trows

    x_d = nc.dram_tensor("x", [rows, DIM], F32, kind="ExternalInput").ap()
    wb_d = nc.dram_tensor("wb", [128, S], F32, kind="ExternalInput").ap()
    bb_d = nc.dram_tensor("bb", [128, S], F32, kind="ExternalInput").ap()
    ndinv_d = nc.dram_tensor("ndinv", [128, Bb * NSEG], F32, kind="ExternalInput").ap()
    out_d = nc.dram_tensor("out", [rows, DIM], F32, kind="ExternalOutput").ap()

    xv = x_d.rearrange("(n p b) f -> n p b f", p=128, b=Bb)
    ov = out_d.rearrange("(n p b) f -> n p b f", p=128, b=Bb)

    def eng(name):
        return {"vector": nc.vector, "gpsimd": nc.gpsimd, "scalar": nc.scalar}[name]

    NS = Bb * NSEG  # 776; stats layout: [v1 | v2 | scal]
    E1 = Bb * G1  # 512
    E2 = E1 + Bb * G2  # 768

    with tile.TileContext(nc) as tc, ExitStack() as ctx:
        const = ctx.enter_context(tc.tile_pool(name="const", bufs=1))
        xpool = ctx.enter_context(tc.tile_pool(name="xp", bufs=3))
        opool = ctx.enter_context(tc.tile_pool(name="op", bufs=3))
        qpool = ctx.enter_context(tc.tile_pool(name="qp", bufs=2))
        stats = ctx.enter_context(tc.tile_pool(name="st", bufs=2))

        wb_t = const.tile([128, S], F32, tag="wb")
        nc.sync.dma_start(wb_t[:], wb_d)
        bb_t = const.tile([128, S], F32, tag="bb")
        nc.sync.dma_start(bb_t[:], bb_d)
        ndinv_t = const.tile([128, NS], F32, tag="ndinv")
        nc.sync.dma_start(ndinv_t[:], ndinv_d)
        wb_b = wb_t[:].rearrange("p (o f) -> p o f", o=1).broadcast_to([128, Bb, S])
        bb_b = bb_t[:].rearrange("p (o f) -> p o f", o=1).broadcast_to([128, Bb, S])

        def emit_load(i):
            st = {}
            xt = xpool.tile([128, Bb * DIM], F32, tag="x")
            nc.sync.dma_start(xt[:], xv[i])
            st["xt"] = xt
            return st

        def emit_sums(st):
            """squares + sums for tile i (VectorE bulk, scal sums on ScalarE)"""
            xt = st["xt"]
            x3 = xt[:].rearrange("p (b f) -> p b f", b=Bb)
            x_1 = x3[:, :, S : S + G1 * D1].rearrange("p b (g d) -> p b g d", d=D1)
            x_2 = x3[:, :, S + G1 * D1 : DIM].rearrange("p b (g d) -> p b g d", d=D2)

            xq = qpool.tile([128, Bb * V12], F32, tag="xq")
            q3 = xq[:].rearrange("p (b f) -> p b f", b=Bb)
            nc.scalar.activation(q3, x3[:, :, S:DIM], AF.Square)
            q_1 = q3[:, :, 0 : G1 * D1].rearrange("p b (g d) -> p b g d", d=D1)
            q_2 = q3[:, :, G1 * D1 : V12].rearrange("p b (g d) -> p b g d", d=D2)

            S1 = stats.tile([128, NS], F32, tag="S1")
            S2 = stats.tile([128, NS], F32, tag="S2")
            s1v1 = S1[:, 0:E1].rearrange("p (b g) -> p b g", b=Bb)
            s1v2 = S1[:, E1:E2].rearrange("p (b g) -> p b g", b=Bb)
            s1s = S1[:, E2:NS]
            s2v1 = S2[:, 0:E1].rearrange("p (b g) -> p b g", b=Bb)
            s2v2 = S2[:, E1:E2].rearrange("p (b g) -> p b g", b=Bb)
            s2s = S2[:, E2:NS]

            t01 = stats.tile([128, E1], F32, tag="t01")
            t013 = t01[:].rearrange("p (b g) -> p b g", b=Bb)
            nc.vector.tensor_add(t013, x_1[:, :, :, 0], x_1[:, :, :, 1])
            nc.vector.tensor_add(s1v1, t013, x_1[:, :, :, 2])
            t01q = stats.tile([128, E1], F32, tag="t01q")
            t01q3 = t01q[:].rearrange("p (b g) -> p b g", b=Bb)
            nc.vector.tensor_add(t01q3, q_1[:, :, :, 0], q_1[:, :, :, 1])
            nc.vector.tensor_add(s2v1, t01q3, q_1[:, :, :, 2])
            nc.vector.reduce_sum(s1v2, x_2, axis=AXX)
            nc.vector.reduce_sum(s2v2, q_2, axis=AXX)

            # scal sums via per-b ScalarE activation+accum; square dump goes
            # to the (dead) out-tile scal region, identity dump to scratch
            ot = opool.tile([128, Bb * DIM], F32, tag="o")
            dump = stats.tile([128, S], F32, tag="dump")
            for b in range(Bb):
                xsb = xt[:, b * DIM : b * DIM + S]
                nc.scalar.activation(
                    ot[:, b * DIM : b * DIM + S], xsb, AF.Square,
                    accum_out=s2s[:, b : b + 1])
                nc.scalar.activation(
                    dump[:], xsb, AF.Identity, accum_out=s1s[:, b : b + 1])

            st.update(x3=x3, x_1=x_1, x_2=x_2, xq=xq, q_1=q_1, q_2=q_2,
                      S1=S1, S2=S2, ot=ot)

        def emit_stats_head(st):
            """nm -> msq -> var(STT) -> rsqrt for tile i (V/S ping-pong,
            emitted early so S/G start while V runs the next tile's sums)"""
            S1, S2 = st["S1"], st["S2"]
            nm = stats.tile([128, NS], F32, tag="nm")
            nc.vector.tensor_mul(nm[:], S1[:], ndinv_t[:])
            msq = stats.tile([128, NS], F32, tag="msq")
            nc.scalar.activation(msq[:], nm[:], AF.Square)
            var = stats.tile([128, NS], F32, tag="var")
            for (lo, hi, d) in ((0, E1, D1), (E1, E2, D2), (E2, NS, S)):
                nc.vector.scalar_tensor_tensor(
                    var[:, lo:hi], S2[:, lo:hi], 1.0 / d, msq[:, lo:hi],
                    op0=ALU.mult, op1=ALU.subtract)
            inv = stats.tile([128, NS], F32, tag="inv")
            _rsqrt(nc, inv[:], var[:], 1.0, EPS)
            st.update(nm=nm, inv=inv)

        def emit_normalize(st):
            """v1/v2 normalize (G + V share) + scal activations (S)"""
            xt, x_1, x_2 = st["xt"], st["x_1"], st["x_2"]
            q_1, q_2 = st["q_1"], st["q_2"]
            nm, inv = st["nm"], st["inv"]

            def bcv(t, lo, g, d):
                return (
                    t[:, lo : lo + Bb * g].rearrange("p (b g) -> p b g", b=Bb)
                    .rearrange("p b (g o) -> p b g o", o=1)
                    .broadcast_to([128, Bb, g, d])
                )
            nm1_b = bcv(nm, 0, G1, D1)
            nm2_b = bcv(nm, E1, G2, D2)
            i1_b = bcv(inv, 0, G1, D1)
            i2_b = bcv(inv, E1, G2, D2)

            ot = st["ot"]
            o3 = ot[:].rearrange("p (b f) -> p b f", b=Bb)
            o_1 = o3[:, :, S : S + G1 * D1].rearrange("p b (g d) -> p b g d", d=D1)
            o_2 = o3[:, :, S + G1 * D1 : DIM].rearrange("p b (g d) -> p b g d", d=D2)
            eng(ENG_C1).tensor_add(q_1, x_1, nm1_b)
            eng(ENG_O1).tensor_mul(o_1, q_1, i1_b)
            eng(ENG_C2).tensor_add(q_2, x_2, nm2_b)
            eng(ENG_O2).tensor_mul(o_2, q_2, i2_b)

            # scal: nmi = nm*inv; per-b t = x*inv[b] + nmi[b] on ScalarE
            nmi = stats.tile([128, Bb], F32, tag="nmi")
            nc.vector.tensor_mul(nmi[:], nm[:, E2:NS], inv[:, E2:NS])
            tscal = stats.tile([128, Bb * S], F32, tag="tscal")
            for b in range(Bb):
                nc.scalar.activation(
                    tscal[:, b * S : (b + 1) * S],
                    xt[:, b * DIM : b * DIM + S],
                    AF.Identity,
                    bias=nmi[:, b : b + 1],
                    scale=inv[:, E2 + b : E2 + b + 1],
                )
            st.update(o3=o3, tscal=tscal)

        def emit_tail(i, st):
            """scal *w + bias (V), then store"""
            o3, tscal = st["o3"], st["tscal"]
            t3 = tscal[:].rearrange("p (b f) -> p b f", b=Bb)
            wv = stats.tile([128, Bb * S], F32, tag="wv")
            wv3 = wv[:].rearrange("p (b f) -> p b f", b=Bb)
            eng(ENG_WM).tensor_mul(wv3, t3, wb_b)
            eng(ENG_BA).tensor_add(o3[:, :, 0:S], wv3, bb_b)
            nc.sync.dma_start(ov[i], st["ot"][:])

        # --- software pipeline ---
        states = {}
        states[0] = emit_load(0)
        if ntiles > 1:
            states[1] = emit_load(1)
        emit_sums(states[0])
        for i in range(ntiles):
            if i + 2 < ntiles:
                states[i + 2] = emit_load(i + 2)
            emit_stats_head(states[i])
            emit_normalize(states[i])
            if i + 1 < ntiles:
                emit_sums(states[i + 1])
            emit_tail(i, states[i])
            states.pop(i - 1, None)

    nc.compile()
    return nc


def _ndinv():
    # stats layout is [v1 (B*G1) | v2 (B*G2) | scal (B)]
    flat = np.concatenate(
        [np.full(B * G1, -1.0 / D1), np.full(B * G2, -1.0 / D2), np.full(B, -1.0 / S)]
    ).astype(np.float32)
    return np.ascontiguousarray(np.broadcast_to(flat, (128, B * NSEG)), np.float32)


def _in_maps(x, weight, bias, rows):
    wb = np.ascontiguousarray(np.broadcast_to(weight, (128, S)), np.float32)
    bb = np.ascontiguousarray(np.broadcast_to(bias, (128, S)), np.float32)
    ndinv = _ndinv()
    return [
        {
            "x": np.ascontiguousarray(x[c * rows : (c + 1) * rows], np.float32),
            "wb": wb,
            "bb": bb,
            "ndinv": ndinv,
        }
        for c in range(N_CORES)
    ]


_NC_CACHE = {}


def kernel(x, weight, bias):
    x = np.asarray(x, np.float32)
    weight = np.asarray(weight, np.float32)
    bias = np.asarray(bias, np.float32)
    key = (x.shape[0] // N_CORES, B)
    if key not in _NC_CACHE:
        _NC_CACHE[key] = build_nc(rows=key[0], b_blocks=B)
    nc = _NC_CACHE[key]
    res = run_bass_kernel_spmd(nc, _in_maps(x, weight, bias, key[0]), list(range(N_CORES)))
    return np.concatenate([res.results[c]["out"] for c in range(N_CORES)], axis=0)
